# revision 19
# baseline (speedup 1.0000x reference)
"""Trainium2 Bass kernel for LinearPerformerAttention (causal linear attention).

Sharding: head-parallel across 8 cores (head c -> core c). Each core computes
its head's causal linear attention over all 2048 tokens via chunked prefix
sums (16 chunks of 128 tokens), then a partial output projection
attn_h @ W_out[h*64:(h+1)*64, :].  The host sums the 8 partial (2048,512)
outputs and adds b_out (tensor-parallel unshard).

Design notes (HW-profiled on trn2; PE is the saturated engine):
  * all-bf16 matmuls (1 cycle/row on PE at any moving-dim size; fp32/f32r
    pay 4x below 256 moving cols), f16 output partials (halves DMA).
  * proj_matrix folded into W_q/W_k on host: qp_pre = x @ (Wq pm), so q/k
    themselves are never formed and the [64,128] projection matmuls vanish.
  * v computed directly token-major from xT k-tiles (no per-chunk transpose).
  * elu1p(t) = min(exp(t), 1 + relu(t)): Exp on ACT, 1+relu on DVE, min on
    DVE (GpSimd cannot run TensorTensor on real HW; ACT/DVE are the only
    PSUM-drain engines).
  * denominator rides the numerator matmul as a 65th row (lhsT = full
    [S|z] / [v|1] tiles) -- saves 2 matmuls+ldweights per chunk; the den row
    returns to a [128,1] column via a 1-row PE transpose so the reciprocal
    runs one-element-per-lane (a [1,128] reciprocal is ~6x slower on HW).
  * PSUM banks (bank-granular slots!): 3 rotate the per-chunk small tiles
    (pt | pa | pv | po65), 2 feature-map psums, 2 outproj psums,
    1 persistent S accumulator. Feature and outproj psums get separate tags
    so phase A of block b+1 never WAR-waits on block b's recursion tails.
  * emission interleaves three streams so the PE never idles: the
    S-recursion of block b-1 sits between the two feature-matmul groups of
    block b, with the x-only-dependent v-matmuls sprinkled between
    recursion steps as pure filler (PE stalls reset its p-state ramp; dense
    PE streams run at a visibly higher clock on HW).
  * DMA: host pre-tiles x ([128, blk, kt, 512]); block 0 loads as 4 k-tile
    DMAs (fast start), blocks 1-3 as one DMA each; weights in 2 DMAs with
    everything feat_q(0)/v need in the first; output staged per 2 chunks
    ([128, 2, 512] f16). HWDGE descriptor generation costs ~625ns per DMA,
    so few big DMAs beat many small ones.
  * 24 standalone LDWEIGHTS in the dead startup window (PE ready ~7us,
    first data ~9.5us). Zero memory side effects (no psum writes, no
    accumulation groups). Measured-neutral-to-slightly-positive; NOTE that
    extending the burst showed the p-state ramp responds to MATMUL
    activity, not LDWEIGHTS, so this is not a reliable clock warm-up.
    Dummy-MATMUL warm-up DID pre-ramp the clock (~2.5us) but exposed a
    timing-sensitive sync hazard on real hardware (deterministic 3.5e-2
    corruption in one configuration, an intermittent NaN in another; the
    instruction-level interpreter reproduces neither) -- do not reintroduce
    without extensive hardware revalidation.
"""

import threading
from contextlib import ExitStack

import numpy as np
import ml_dtypes

import concourse.bass as bass
import concourse.mybir as mybir
import concourse.tile as tile
from concourse import bacc
from concourse.bass_utils import run_bass_kernel_spmd

DIM, HEADS, FEAT = 512, 8, 128
HD = DIM // HEADS          # 64
N = 2048
C = 128                    # chunk (tokens)
NCH = N // C               # 16
NBLK = 4                   # token blocks of 512 for phase A
KT = 4                     # k-tiles of 128 over DIM

F32 = mybir.dt.float32
BF16 = mybir.dt.bfloat16
F16 = mybir.dt.float16
AF = mybir.ActivationFunctionType
ALU = mybir.AluOpType

NP_BF16 = ml_dtypes.bfloat16

# wcat column layout: [wqp(4*128) | wv(4*64) | wkp(4*128) | mask(128) | id(128)]
# first DMA covers wqp+wv (everything feat_q(0) and the v-matmuls need)
WQP0 = 0
WV0 = KT * FEAT                 # 512
WKP0 = WV0 + KT * HD            # 768
WCAT1 = WKP0                    # first-DMA column count
MASK0 = WKP0 + KT * FEAT        # 1280
ID0 = MASK0 + 128               # 1408
WCAT_COLS = ID0 + 128           # 1536



def build_nc():
    nc = bacc.Bacc()

    xT_d = nc.declare_dram_parameter("xt", [128, NBLK, KT, 512], BF16,
                                     isOutput=False)
    wcat_d = nc.declare_dram_parameter("wcat", [128, WCAT_COLS], BF16,
                                       isOutput=False)
    wo_d = nc.declare_dram_parameter("w_out_h", [HD, DIM], BF16, isOutput=False)
    # out[p, c, col] = full_out[token = c*128 + p, col]
    out_d = nc.declare_dram_parameter("out_part", [128, NCH, DIM], F16,
                                      isOutput=True)

    with ExitStack() as ctx:
        tc = ctx.enter_context(tile.TileContext(nc))
        const = ctx.enter_context(tc.tile_pool(name="const", bufs=1))
        fpool = ctx.enter_context(tc.tile_pool(name="feat", bufs=3))
        spool = ctx.enter_context(tc.tile_pool(name="spool", bufs=3))
        ampool = ctx.enter_context(tc.tile_pool(name="am", bufs=8))
        numpool = ctx.enter_context(tc.tile_pool(name="num", bufs=3))
        opool = ctx.enter_context(tc.tile_pool(name="osb", bufs=3))
        dpool = ctx.enter_context(tc.tile_pool(name="dinv", bufs=3))
        # PSUM banks: psml(3) + feat(2) + pj(2) + psp(1) = 8
        psml = ctx.enter_context(tc.tile_pool(name="psml", bufs=3, space="PSUM"))
        pbig = ctx.enter_context(tc.tile_pool(name="pbig", bufs=2, space="PSUM"))
        psp = ctx.enter_context(tc.tile_pool(name="psp", bufs=1, space="PSUM"))

        ps_s = psp.tile([FEAT, HD + 1], F32)   # persistent S accumulator

        # ---- constants; DMA order = first-use order so feat(0) starts early:
        # wqp, x-block0 k-tiles, wkp+wv+mask+id, x-blocks 1-3, wo ----
        wcat = const.tile([128, WCAT_COLS], BF16)
        wo_sb = const.tile([HD, DIM], BF16)
        nc.sync.dma_start(wcat[:, 0:WCAT1], wcat_d[:, 0:WCAT1])

        def wqp(kk):
            return wcat[:, WQP0 + kk * FEAT: WQP0 + (kk + 1) * FEAT]

        def wkp(kk):
            return wcat[:, WKP0 + kk * FEAT: WKP0 + (kk + 1) * FEAT]

        def wv(kk):
            return wcat[:, WV0 + kk * HD: WV0 + (kk + 1) * HD]

        mask_sb = wcat[:, MASK0:MASK0 + 128]
        id_sb = wcat[:, ID0:ID0 + 128]

        s0_sb = const.tile([FEAT, HD + 1], BF16)
        nc.vector.memset(s0_sb[:], 0.0)
        id_one = id_sb[HD:HD + 1, HD:HD + 1]   # [[1]] at base partition 64
        dw = const.tile([128, 128], BF16)      # zeros: p-state warm-up operand
        nc.vector.memset(dw[:], 0.0)

        # ---- persistent intermediates ----
        xsb = const.tile([128, NBLK, KT, 512], BF16)   # all of x^T, pre-tiled
        qpT = const.tile([FEAT, N], BF16)              # elu1p(x @ (Wq pm))^T
        kpT = const.tile([FEAT, N], BF16)
        kp_tm = const.tile([128, N], BF16)             # token-major kp
        vv = const.tile([128, NCH, HD + 1], BF16)      # [v | 1] per chunk
        nc.vector.memset(vv[:, :, HD:HD + 1], 1.0)     # ones column, all chunks

        # x DMAs up front; block 0 split by k-tile so featmuls start early.
        # k-tiles 0-1 go through the scalar queue (2nd HWDGE ring) so their
        # ~0.7us descriptor gens run in parallel with sync's wcat1.
        # (NEVER gpsimd SWDGE here: its SBUF descriptor rings + 16 SDMA
        # fetchers on partitions 0-31 slow every SBUF op ~20%, HW-measured.)
        nc.scalar.dma_start(xsb[:, 0, 0, :], xT_d[:, 0, 0, :])
        nc.scalar.dma_start(xsb[:, 0, 1, :], xT_d[:, 0, 1, :])
        nc.sync.dma_start(xsb[:, 0, 2, :], xT_d[:, 0, 2, :])
        nc.sync.dma_start(xsb[:, 0, 3, :], xT_d[:, 0, 3, :])
        nc.sync.dma_start(wcat[:, WCAT1:], wcat_d[:, WCAT1:])
        for blk in range(1, NBLK):
            nc.sync.dma_start(xsb[:, blk, :, :], xT_d[:, blk, :, :])
        nc.sync.dma_start(wo_sb[:], wo_d[:])

        # ---- emission helpers ----
        def feat_half(blk, which):
            sl = slice(blk * 512, (blk + 1) * 512)
            wfn, dstT = ((wqp, qpT), (wkp, kpT))[which]
            ps = pbig.tile([FEAT, 512], F32, tag="fps", name=f"fps{blk}_{which}")
            for kk in range(KT):
                nc.tensor.matmul(ps[:], wfn(kk), xsb[:, blk, kk, :],
                                 start=(kk == 0), stop=(kk == KT - 1))
            e = fpool.tile([FEAT, 512], BF16, tag="e", name=f"e{blk}_{which}")
            nc.scalar.activation(e[:], ps[:], AF.Exp)
            r = fpool.tile([FEAT, 512], BF16, tag="r", name=f"r{blk}_{which}")
            nc.vector.tensor_scalar(r[:], ps[:], 0.0, 1.0, ALU.max, ALU.add)
            nc.vector.tensor_tensor(dstT[:, sl], e[:], r[:], ALU.min)

        am_tiles = {}

        def v_chunk(i):
            """Token-major v for chunk i, straight from xT k-tiles. Depends
            only on the x DMA -- used as PE filler between recursion steps."""
            blk, sub = i // 4, i % 4
            cs = slice(sub * C, (sub + 1) * C)
            pv = psml.tile([128, HD], F32, tag="sml", name=f"pv{i}")
            for kk in range(KT):
                nc.tensor.matmul(pv[:], xsb[:, blk, kk, cs], wv(kk),
                                 start=(kk == 0), stop=(kk == KT - 1))
            # ACT copy (not DVE): DVE is the phase-A bottleneck engine
            nc.scalar.activation(vv[:, i, 0:HD], pv[:], AF.Copy)

        def prep_chunk(i):
            """S-independent per-chunk work: kp transpose, masked A^T."""
            ci = slice(i * C, (i + 1) * C)
            pt = psml.tile([128, 128], BF16, tag="sml", name=f"pt{i}")
            nc.tensor.transpose(pt[:], kpT[:, ci], id_sb)
            nc.vector.tensor_copy(kp_tm[:, ci], pt[:])
            pa = psml.tile([128, 128], F32, tag="sml", name=f"pa{i}")
            nc.tensor.matmul(pa[:], kpT[:, ci], qpT[:, ci], start=True, stop=True)
            am = ampool.tile([128, 128], BF16, name=f"am{i}")
            nc.vector.tensor_tensor(am[:], pa[:], mask_sb, ALU.mult)
            am_tiles[i] = am

        osb2 = [None]

        def emit_tail(num, dinv, i):
            pj = pbig.tile([128, DIM], F32, tag="pj", name=f"pj{i}")
            nc.tensor.matmul(pj[:], num[0:HD, :], wo_sb[:], start=True, stop=True)
            if i >= NCH - 2:
                # last two chunks ship individually: chunk 14's DMA starts a
                # drain earlier and the final transfer is only 128KB
                osb = opool.tile([128, 1, DIM], F16, name=f"osb1_{i}")
                nc.scalar.activation(osb[:, 0, :], pj[:], AF.Copy,
                                     scale=dinv[:])
                nc.sync.dma_start(out_d[:, i:i + 1, :], osb[:])
                return
            if i % 2 == 0:
                osb2[0] = opool.tile([128, 2, DIM], F16, name=f"osb2_{i}")
            osb = osb2[0]
            nc.scalar.activation(osb[:, i % 2, :], pj[:], AF.Copy,
                                 scale=dinv[:])
            if i % 2 == 1:
                nc.sync.dma_start(out_d[:, i - 1:i + 1, :], osb[:])

        state = {"s_prev": s0_sb, "pending": None}

        def rec_chunk(i):
            """S-chain per-chunk work + pipelined tail of chunk i-1."""
            blk, sub = i // 4, i % 4
            ci = slice(i * C, (i + 1) * C)
            cs = slice(sub * C, (sub + 1) * C)
            s_prev = state["s_prev"]
            am = am_tiles.pop(i)
            # numerator psum tile; row 64 = denominator^T
            pon = psml.tile([HD + 1, 128], F32, tag="sml", name=f"pk{i}")
            po65 = pon[:]
            # S' += kp_tm^T @ [v|1]  (PSUM accumulation across chunks)
            nc.tensor.matmul(ps_s[:], kp_tm[:, ci], vv[:, i, :],
                             start=(i == 0), stop=(i == NCH - 1),
                             skip_group_check=True)
            # [num^T; den^T] [65, ti] = [S|z]^T qpc + [v|1]^T am
            nc.tensor.matmul(po65, s_prev[:], qpT[:, ci],
                             start=True, stop=False, skip_group_check=True)
            nc.tensor.matmul(po65, vv[:, i, :], am[:],
                             start=False, stop=True, skip_group_check=True)
            # snapshot S for next iter (DVE: keeps the S-chain off the ACT
            # queue, which is busy with the 570ns output drains)
            s_new = spool.tile([FEAT, HD + 1], BF16, name=f"s{i}")
            nc.vector.tensor_copy(s_new[:], ps_s[:])
            # num copy brings the den row along (row 64, bf16)
            num = numpool.tile([HD + 1, 128], BF16, name=f"num{i}")
            nc.vector.tensor_copy(num[:], po65)
            # previous chunk's outproj here: covers num-copy latency
            if state["pending"] is not None:
                emit_tail(*state["pending"])
            # den row -> column (tiny bf16 transpose), then [128,1] reciprocal
            pdc = psml.tile([128, 1], BF16, tag="sml", name=f"pdc{i}")
            nc.tensor.transpose(pdc[:], num[HD:HD + 1, :], id_one)
            dinv = dpool.tile([128, 1], F32, name=f"dinv{i}")
            nc.vector.reciprocal(dinv[:], pdc[:])
            state["pending"] = (num, dinv, i)
            state["s_prev"] = s_new

        # ---- p-state warm-up: the PE idles ~4us waiting for the first
        # weight/x DMAs, and its clock ramps only after sustained busy.
        # Standalone LDWEIGHTS bursts occupy the PE pipeline with ZERO
        # memory side effects (no psum writes, no accumulation groups) --
        # unlike dummy matmuls, which exposed a timing-sensitive sync
        # hazard (deterministic 3.5e-2 / intermittent NaN corruption). ----
        for dmy in range(24):
            nc.tensor.ldweights(dw[:])

        # ---- emission schedule: v-matmuls of the next block are pure
        # filler (x-DMA dep only) sprinkled between recursion steps so the
        # PE never idles while DVE/ACT drain the S-chain tiles ----
        feat_half(0, 0)
        for sub in range(4):
            v_chunk(sub)
        feat_half(0, 1)
        for sub in range(4):
            prep_chunk(sub)
        for blk in range(1, NBLK):
            feat_half(blk, 0)
            rec_chunk(4 * (blk - 1) + 0)
            v_chunk(4 * blk + 0)
            rec_chunk(4 * (blk - 1) + 1)
            v_chunk(4 * blk + 1)
            feat_half(blk, 1)
            rec_chunk(4 * (blk - 1) + 2)
            v_chunk(4 * blk + 2)
            rec_chunk(4 * (blk - 1) + 3)
            v_chunk(4 * blk + 3)
            if blk < NBLK - 1:
                for sub in range(4):
                    prep_chunk(4 * blk + sub)
        # tail: weave the last block's preps between the bare rec steps so
        # the PE has independent work while the S-chain round-trips
        prep_chunk(12)
        prep_chunk(13)
        rec_chunk(12)
        prep_chunk(14)
        rec_chunk(13)
        prep_chunk(15)
        rec_chunk(14)
        rec_chunk(15)
        emit_tail(*state["pending"])

    nc.compile()
    return nc


_cache = threading.Lock()
_nc = None


def _get_nc():
    global _nc
    with _cache:
        if _nc is None:
            _nc = build_nc()
    return _nc


def _in_maps(x, proj_matrix, W_qkv, W_out):
    # x^T pre-tiled: [512, 2048] -> [kt, 128, blk, 512] -> [128, blk, kt, 512]
    xT = np.ascontiguousarray(
        x[0].T.reshape(KT, 128, NBLK, 512).transpose(1, 2, 0, 3)
    ).astype(NP_BF16)
    mask = (np.arange(128)[:, None] <= np.arange(128)[None, :]).astype(np.float32)
    ident = np.eye(128, dtype=np.float32)

    def ktile(cols):
        # (512, m) -> (128, 4*m) k-tile layout, kt-major columns
        m = cols.shape[1]
        return cols.reshape(KT, 128, m).transpose(1, 0, 2).reshape(128, KT * m)

    maps = []
    for c in range(HEADS):
        pm = proj_matrix[c]                                 # (64, 128)
        wq = W_qkv[:, c * HD:(c + 1) * HD]                  # (512, 64)
        wk = W_qkv[:, DIM + c * HD: DIM + (c + 1) * HD]
        wv_ = W_qkv[:, 2 * DIM + c * HD: 2 * DIM + (c + 1) * HD]
        wcat = np.concatenate(
            [ktile(wq @ pm), ktile(wv_), ktile(wk @ pm), mask, ident],
            axis=1).astype(NP_BF16)
        maps.append({
            "xt": xT,
            "wcat": np.ascontiguousarray(wcat),
            "w_out_h": np.ascontiguousarray(
                W_out[c * HD:(c + 1) * HD, :]).astype(NP_BF16),
        })
    return maps


def kernel(x, proj_matrix, W_qkv, W_out, b_out, _trace=False):
    x = np.asarray(x, dtype=np.float32)
    proj_matrix = np.asarray(proj_matrix, dtype=np.float32)
    W_qkv = np.asarray(W_qkv, dtype=np.float32)
    W_out = np.asarray(W_out, dtype=np.float32)
    b_out = np.asarray(b_out, dtype=np.float32)

    nc = _get_nc()
    maps = _in_maps(x, proj_matrix, W_qkv, W_out)
    res = run_bass_kernel_spmd(nc, maps, core_ids=list(range(HEADS)), trace=_trace)
    out = np.zeros((N, DIM), dtype=np.float32)
    for r in res.results:
        part = np.asarray(r["out_part"], dtype=np.float32)   # [128, 16, 512]
        out += part.transpose(1, 0, 2).reshape(N, DIM)
    out += b_out
    if _trace:
        return out.reshape(1, N, DIM), res
    return out.reshape(1, N, DIM)



# revision 21
# speedup vs baseline: 1.0248x; 1.0248x over previous
"""Trainium2 Bass kernel for LinearPerformerAttention (causal linear attention).

Sharding: head-parallel across 8 cores (head c -> core c). Each core computes
its head's causal linear attention over all 2048 tokens via chunked prefix
sums (16 chunks of 128 tokens), then a partial output projection
attn_h @ W_out[h*64:(h+1)*64, :].  The host sums the 8 partial (2048,512)
outputs and adds b_out (tensor-parallel unshard).

Design notes (HW-profiled on trn2; PE is the saturated engine):
  * all-bf16 matmuls (1 cycle/row on PE at any moving-dim size; fp32/f32r
    pay 4x below 256 moving cols), f16 output partials (halves DMA).
  * proj_matrix folded into W_q/W_k on host: qp_pre = x @ (Wq pm), so q/k
    themselves are never formed and the [64,128] projection matmuls vanish.
  * v computed directly token-major from xT k-tiles (no per-chunk transpose).
  * elu1p(t) = min(exp(t), 1 + relu(t)): Exp on ACT, 1+relu on DVE, min on
    DVE (GpSimd cannot run TensorTensor on real HW; ACT/DVE are the only
    PSUM-drain engines).
  * denominator rides the numerator matmul as a 65th row (lhsT = full
    [S|z] / [v|1] tiles) -- saves 2 matmuls+ldweights per chunk; the den row
    returns to a [128,1] column via a 1-row PE transpose so the reciprocal
    runs one-element-per-lane (a [1,128] reciprocal is ~6x slower on HW).
  * PSUM banks (bank-granular slots!): 3 rotate the per-chunk small tiles
    (pt | pa | pv | po65), 2 feature-map psums, 2 outproj psums,
    1 persistent S accumulator. Feature and outproj psums get separate tags
    so phase A of block b+1 never WAR-waits on block b's recursion tails.
  * emission interleaves three streams so the PE never idles: the
    S-recursion of block b-1 sits between the two feature-matmul groups of
    block b, with the x-only-dependent v-matmuls sprinkled between
    recursion steps as pure filler (PE stalls reset its p-state ramp; dense
    PE streams run at a visibly higher clock on HW).
  * DMA: host pre-tiles x ([128, blk, kt, 512]); block 0 loads as 4 k-tile
    DMAs (fast start), blocks 1-3 as one DMA each; weights in 2 DMAs with
    everything feat_q(0)/v need in the first; output staged per 2 chunks
    ([128, 2, 512] f16). HWDGE descriptor generation costs ~625ns per DMA,
    so few big DMAs beat many small ones.
  * 24 standalone LDWEIGHTS in the dead startup window (PE ready ~7us,
    first data ~9.5us). Zero memory side effects (no psum writes, no
    accumulation groups). Measured-neutral-to-slightly-positive; NOTE that
    extending the burst showed the p-state ramp responds to MATMUL
    activity, not LDWEIGHTS, so this is not a reliable clock warm-up.
    Dummy-MATMUL warm-up DID pre-ramp the clock (~2.5us) but exposed a
    timing-sensitive sync hazard on real hardware (deterministic 3.5e-2
    corruption in one configuration, an intermittent NaN in another; the
    instruction-level interpreter reproduces neither) -- do not reintroduce
    without extensive hardware revalidation.
"""

import threading
from contextlib import ExitStack

import numpy as np
import ml_dtypes

import concourse.bass as bass
import concourse.mybir as mybir
import concourse.tile as tile
from concourse import bacc
from concourse.bass_utils import run_bass_kernel_spmd

DIM, HEADS, FEAT = 512, 8, 128
HD = DIM // HEADS          # 64
N = 2048
C = 128                    # chunk (tokens)
NCH = N // C               # 16
NBLK = 4                   # token blocks of 512 for phase A
KT = 4                     # k-tiles of 128 over DIM

F32 = mybir.dt.float32
BF16 = mybir.dt.bfloat16
F16 = mybir.dt.float16
AF = mybir.ActivationFunctionType
ALU = mybir.AluOpType

NP_BF16 = ml_dtypes.bfloat16

# wcat column layout: [wqp(4*128) | wv(4*64) | wkp(4*128) | mask(128) | id(128)]
# first DMA covers wqp+wv (everything feat_q(0) and the v-matmuls need)
WQP0 = 0
WV0 = KT * FEAT                 # 512
WKP0 = WV0 + KT * HD            # 768
WCAT1 = WKP0                    # first-DMA column count
MASK0 = WKP0 + KT * FEAT        # 1280
ID0 = MASK0 + 128               # 1408
WCAT_COLS = ID0 + 128           # 1536



def build_nc():
    nc = bacc.Bacc()

    xT_d = nc.declare_dram_parameter("xt", [128, NBLK, KT, 512], BF16,
                                     isOutput=False)
    wcat_d = nc.declare_dram_parameter("wcat", [128, WCAT_COLS], BF16,
                                       isOutput=False)
    wo_d = nc.declare_dram_parameter("w_out_h", [HD, DIM], BF16, isOutput=False)
    # out[p, c, col] = full_out[token = c*128 + p, col]
    out_d = nc.declare_dram_parameter("out_part", [128, NCH, DIM], F16,
                                      isOutput=True)

    with ExitStack() as ctx:
        tc = ctx.enter_context(tile.TileContext(nc))
        const = ctx.enter_context(tc.tile_pool(name="const", bufs=1))
        fpool = ctx.enter_context(tc.tile_pool(name="feat", bufs=3))
        spool = ctx.enter_context(tc.tile_pool(name="spool", bufs=3))
        ampool = ctx.enter_context(tc.tile_pool(name="am", bufs=8))
        numpool = ctx.enter_context(tc.tile_pool(name="num", bufs=3))
        opool = ctx.enter_context(tc.tile_pool(name="osb", bufs=3))
        dpool = ctx.enter_context(tc.tile_pool(name="dinv", bufs=3))
        # PSUM banks: psml(3) + feat(2) + pj(2) + psp(1) = 8
        psml = ctx.enter_context(tc.tile_pool(name="psml", bufs=3, space="PSUM"))
        pbig = ctx.enter_context(tc.tile_pool(name="pbig", bufs=2, space="PSUM"))
        psp = ctx.enter_context(tc.tile_pool(name="psp", bufs=1, space="PSUM"))

        ps_s = psp.tile([FEAT, HD + 1], F32)   # persistent S accumulator

        # ---- constants; DMA order = first-use order so feat(0) starts early:
        # wqp, x-block0 k-tiles, wkp+wv+mask+id, x-blocks 1-3, wo ----
        wcat = const.tile([128, WCAT_COLS], BF16)
        wo_sb = const.tile([HD, DIM], BF16)
        nc.sync.dma_start(wcat[:, 0:WCAT1], wcat_d[:, 0:WCAT1])

        def wqp(kk):
            return wcat[:, WQP0 + kk * FEAT: WQP0 + (kk + 1) * FEAT]

        def wkp(kk):
            return wcat[:, WKP0 + kk * FEAT: WKP0 + (kk + 1) * FEAT]

        def wv(kk):
            return wcat[:, WV0 + kk * HD: WV0 + (kk + 1) * HD]

        mask_sb = wcat[:, MASK0:MASK0 + 128]
        id_sb = wcat[:, ID0:ID0 + 128]

        s0_sb = const.tile([FEAT, HD + 1], BF16)
        nc.vector.memset(s0_sb[:], 0.0)
        id_one = id_sb[HD:HD + 1, HD:HD + 1]   # [[1]] at base partition 64

        # ---- persistent intermediates ----
        xsb = const.tile([128, NBLK, KT, 512], BF16)   # all of x^T, pre-tiled
        qpT = const.tile([FEAT, N], BF16)              # elu1p(x @ (Wq pm))^T
        kpT = const.tile([FEAT, N], BF16)
        kp_tm = const.tile([128, N], BF16)             # token-major kp
        vv = const.tile([128, NCH, HD + 1], BF16)      # [v | 1] per chunk
        nc.vector.memset(vv[:, :, HD:HD + 1], 1.0)     # ones column, all chunks

        # x DMAs up front; block 0 split by k-tile so featmuls start early.
        # k-tiles 0-1 go through the scalar queue (2nd HWDGE ring) so their
        # ~0.7us descriptor gens run in parallel with sync's wcat1.
        # (NEVER gpsimd SWDGE here: its SBUF descriptor rings + 16 SDMA
        # fetchers on partitions 0-31 slow every SBUF op ~20%, HW-measured.)
        nc.scalar.dma_start(xsb[:, 0, 0, :], xT_d[:, 0, 0, :])
        nc.scalar.dma_start(xsb[:, 0, 1, :], xT_d[:, 0, 1, :])
        nc.sync.dma_start(xsb[:, 0, 2, :], xT_d[:, 0, 2, :])
        nc.sync.dma_start(xsb[:, 0, 3, :], xT_d[:, 0, 3, :])
        nc.sync.dma_start(wcat[:, WCAT1:], wcat_d[:, WCAT1:])
        for blk in range(1, NBLK):
            nc.sync.dma_start(xsb[:, blk, :, :], xT_d[:, blk, :, :])
        nc.sync.dma_start(wo_sb[:], wo_d[:])

        # ---- emission helpers ----
        def feat_half(blk, which):
            sl = slice(blk * 512, (blk + 1) * 512)
            wfn, dstT = ((wqp, qpT), (wkp, kpT))[which]
            ps = pbig.tile([FEAT, 512], F32, tag="fps", name=f"fps{blk}_{which}")
            for kk in range(KT):
                nc.tensor.matmul(ps[:], wfn(kk), xsb[:, blk, kk, :],
                                 start=(kk == 0), stop=(kk == KT - 1))
            e = fpool.tile([FEAT, 512], BF16, tag="e", name=f"e{blk}_{which}")
            nc.scalar.activation(e[:], ps[:], AF.Exp)
            r = fpool.tile([FEAT, 512], BF16, tag="r", name=f"r{blk}_{which}")
            nc.vector.tensor_scalar(r[:], ps[:], 0.0, 1.0, ALU.max, ALU.add)
            nc.vector.tensor_tensor(dstT[:, sl], e[:], r[:], ALU.min)

        am_tiles = {}

        def v_chunk(i):
            """Token-major v for chunk i, straight from xT k-tiles. Depends
            only on the x DMA -- used as PE filler between recursion steps."""
            blk, sub = i // 4, i % 4
            cs = slice(sub * C, (sub + 1) * C)
            pv = psml.tile([128, HD], F32, tag="sml", name=f"pv{i}")
            for kk in range(KT):
                nc.tensor.matmul(pv[:], xsb[:, blk, kk, cs], wv(kk),
                                 start=(kk == 0), stop=(kk == KT - 1))
            # ACT copy (not DVE): DVE is the phase-A bottleneck engine
            nc.scalar.activation(vv[:, i, 0:HD], pv[:], AF.Copy)

        def prep_chunk(i):
            """S-independent per-chunk work: kp transpose, masked A^T."""
            ci = slice(i * C, (i + 1) * C)
            pt = psml.tile([128, 128], BF16, tag="sml", name=f"pt{i}")
            nc.tensor.transpose(pt[:], kpT[:, ci], id_sb)
            nc.vector.tensor_copy(kp_tm[:, ci], pt[:])
            pa = psml.tile([128, 128], F32, tag="sml", name=f"pa{i}")
            nc.tensor.matmul(pa[:], kpT[:, ci], qpT[:, ci], start=True, stop=True)
            am = ampool.tile([128, 128], BF16, name=f"am{i}")
            nc.vector.tensor_tensor(am[:], pa[:], mask_sb, ALU.mult)
            am_tiles[i] = am

        osb2 = [None]

        def emit_tail(num, dinv, i):
            pj = pbig.tile([128, DIM], F32, tag="pj", name=f"pj{i}")
            nc.tensor.matmul(pj[:], num[0:HD, :], wo_sb[:], start=True, stop=True)
            if i >= NCH - 2:
                # last two chunks ship individually: chunk 14's DMA starts a
                # drain earlier and the final transfer is only 128KB
                osb = opool.tile([128, 1, DIM], F16, name=f"osb1_{i}")
                nc.scalar.activation(osb[:, 0, :], pj[:], AF.Copy,
                                     scale=dinv[:])
                nc.sync.dma_start(out_d[:, i:i + 1, :], osb[:])
                return
            if i % 2 == 0:
                osb2[0] = opool.tile([128, 2, DIM], F16, name=f"osb2_{i}")
            osb = osb2[0]
            nc.scalar.activation(osb[:, i % 2, :], pj[:], AF.Copy,
                                 scale=dinv[:])
            if i % 2 == 1:
                nc.sync.dma_start(out_d[:, i - 1:i + 1, :], osb[:])

        state = {"s_prev": s0_sb, "pending": None}

        def rec_chunk(i):
            """S-chain per-chunk work + pipelined tail of chunk i-1."""
            blk, sub = i // 4, i % 4
            ci = slice(i * C, (i + 1) * C)
            cs = slice(sub * C, (sub + 1) * C)
            s_prev = state["s_prev"]
            am = am_tiles.pop(i)
            # numerator psum tile; row 64 = denominator^T
            pon = psml.tile([HD + 1, 128], F32, tag="sml", name=f"pk{i}")
            po65 = pon[:]
            # S' += kp_tm^T @ [v|1]  (PSUM accumulation across chunks)
            nc.tensor.matmul(ps_s[:], kp_tm[:, ci], vv[:, i, :],
                             start=(i == 0), stop=(i == NCH - 1),
                             skip_group_check=True)
            # [num^T; den^T] [65, ti] = [S|z]^T qpc + [v|1]^T am
            nc.tensor.matmul(po65, s_prev[:], qpT[:, ci],
                             start=True, stop=False, skip_group_check=True)
            nc.tensor.matmul(po65, vv[:, i, :], am[:],
                             start=False, stop=True, skip_group_check=True)
            # snapshot S for next iter (DVE: keeps the S-chain off the ACT
            # queue, which is busy with the 570ns output drains)
            s_new = spool.tile([FEAT, HD + 1], BF16, name=f"s{i}")
            nc.vector.tensor_copy(s_new[:], ps_s[:])
            # num copy brings the den row along (row 64, bf16)
            num = numpool.tile([HD + 1, 128], BF16, name=f"num{i}")
            nc.vector.tensor_copy(num[:], po65)
            # previous chunk's outproj here: covers num-copy latency
            if state["pending"] is not None:
                emit_tail(*state["pending"])
            # den row -> column (tiny bf16 transpose), then [128,1] reciprocal
            pdc = psml.tile([128, 1], BF16, tag="sml", name=f"pdc{i}")
            nc.tensor.transpose(pdc[:], num[HD:HD + 1, :], id_one)
            dinv = dpool.tile([128, 1], F32, name=f"dinv{i}")
            nc.vector.reciprocal(dinv[:], pdc[:])
            state["pending"] = (num, dinv, i)
            state["s_prev"] = s_new

        # ---- emission schedule (lag-2): the S-recursion of chunk i runs
        # two chunks behind its prep, so only TWO bare rec steps remain at
        # the end (the bare-rec tail is where the PE goes gappy and the HAM
        # clock-gate re-throttles to 1.2GHz -- measured 537ns N=512 MMs).
        # v/feat/prep matmuls are interleaved as filler between rec steps
        # so the PE stays dense while DVE/ACT service the S-chain. ----
        feat_half(0, 0)
        for sub in range(4):
            v_chunk(sub)
        feat_half(0, 1)
        prep_chunk(0)
        prep_chunk(1)
        rec_chunk(0)
        prep_chunk(2)
        rec_chunk(1)
        prep_chunk(3)
        for blk in range(1, NBLK):
            feat_half(blk, 0)
            rec_chunk(4 * blk - 2)
            v_chunk(4 * blk + 0)
            rec_chunk(4 * blk - 1)
            v_chunk(4 * blk + 1)
            feat_half(blk, 1)
            prep_chunk(4 * blk + 0)
            prep_chunk(4 * blk + 1)
            rec_chunk(4 * blk + 0)
            v_chunk(4 * blk + 2)
            rec_chunk(4 * blk + 1)
            v_chunk(4 * blk + 3)
            prep_chunk(4 * blk + 2)
            prep_chunk(4 * blk + 3)
        rec_chunk(14)
        rec_chunk(15)
        emit_tail(*state["pending"])

    nc.compile()
    return nc


_cache = threading.Lock()
_nc = None


def _get_nc():
    global _nc
    with _cache:
        if _nc is None:
            _nc = build_nc()
    return _nc


def _in_maps(x, proj_matrix, W_qkv, W_out):
    # x^T pre-tiled: [512, 2048] -> [kt, 128, blk, 512] -> [128, blk, kt, 512]
    xT = np.ascontiguousarray(
        x[0].T.reshape(KT, 128, NBLK, 512).transpose(1, 2, 0, 3)
    ).astype(NP_BF16)
    mask = (np.arange(128)[:, None] <= np.arange(128)[None, :]).astype(np.float32)
    ident = np.eye(128, dtype=np.float32)

    def ktile(cols):
        # (512, m) -> (128, 4*m) k-tile layout, kt-major columns
        m = cols.shape[1]
        return cols.reshape(KT, 128, m).transpose(1, 0, 2).reshape(128, KT * m)

    maps = []
    for c in range(HEADS):
        pm = proj_matrix[c]                                 # (64, 128)
        wq = W_qkv[:, c * HD:(c + 1) * HD]                  # (512, 64)
        wk = W_qkv[:, DIM + c * HD: DIM + (c + 1) * HD]
        wv_ = W_qkv[:, 2 * DIM + c * HD: 2 * DIM + (c + 1) * HD]
        wcat = np.concatenate(
            [ktile(wq @ pm), ktile(wv_), ktile(wk @ pm), mask, ident],
            axis=1).astype(NP_BF16)
        maps.append({
            "xt": xT,
            "wcat": np.ascontiguousarray(wcat),
            "w_out_h": np.ascontiguousarray(
                W_out[c * HD:(c + 1) * HD, :]).astype(NP_BF16),
        })
    return maps


def kernel(x, proj_matrix, W_qkv, W_out, b_out, _trace=False):
    x = np.asarray(x, dtype=np.float32)
    proj_matrix = np.asarray(proj_matrix, dtype=np.float32)
    W_qkv = np.asarray(W_qkv, dtype=np.float32)
    W_out = np.asarray(W_out, dtype=np.float32)
    b_out = np.asarray(b_out, dtype=np.float32)

    nc = _get_nc()
    maps = _in_maps(x, proj_matrix, W_qkv, W_out)
    res = run_bass_kernel_spmd(nc, maps, core_ids=list(range(HEADS)), trace=_trace)
    out = np.zeros((N, DIM), dtype=np.float32)
    for r in res.results:
        part = np.asarray(r["out_part"], dtype=np.float32)   # [128, 16, 512]
        out += part.transpose(1, 0, 2).reshape(N, DIM)
    out += b_out
    if _trace:
        return out.reshape(1, N, DIM), res
    return out.reshape(1, N, DIM)



# revision 24
# speedup vs baseline: 1.0678x; 1.0420x over previous
"""Trainium2 Bass kernel for LinearPerformerAttention (causal linear attention).

Sharding: head-parallel across 8 cores (head c -> core c). Each core computes
its head's causal linear attention over all 2048 tokens via chunked prefix
sums (16 chunks of 128 tokens), then a partial output projection
attn_h @ W_out[h*64:(h+1)*64, :].  The host sums the 8 partial (2048,512)
outputs and adds b_out (tensor-parallel unshard).

Design notes (HW-profiled on trn2; PE is the saturated engine):
  * all-bf16 matmuls (1 cycle/row on PE at any moving-dim size; fp32/f32r
    pay 4x below 256 moving cols), f16 output partials (halves DMA).
  * proj_matrix folded into W_q/W_k on host: qp_pre = x @ (Wq pm), so q/k
    themselves are never formed and the [64,128] projection matmuls vanish.
  * v computed directly token-major from xT k-tiles (no per-chunk transpose).
  * elu1p(t) = min(exp(t), 1 + relu(t)): Exp on ACT, 1+relu on DVE, min on
    DVE (GpSimd cannot run TensorTensor on real HW; ACT/DVE are the only
    PSUM-drain engines).
  * denominator rides the numerator matmul as a 65th row (lhsT = full
    [S|z] / [v|1] tiles) -- saves 2 matmuls+ldweights per chunk; the den row
    returns to a [128,1] column via a 1-row PE transpose so the reciprocal
    runs one-element-per-lane (a [1,128] reciprocal is ~6x slower on HW).
  * PSUM banks (bank-granular slots!): 3 rotate the per-chunk small tiles
    (pt | pa | pv | po65), 2 feature-map psums, 2 outproj psums,
    1 persistent S accumulator. Feature and outproj psums get separate tags
    so phase A of block b+1 never WAR-waits on block b's recursion tails.
  * emission interleaves three streams so the PE never idles: the
    S-recursion of block b-1 sits between the two feature-matmul groups of
    block b, with the x-only-dependent v-matmuls sprinkled between
    recursion steps as pure filler (PE stalls reset its p-state ramp; dense
    PE streams run at a visibly higher clock on HW).
  * DMA: host pre-tiles x ([128, blk, kt, 512]); block 0 loads as 4 k-tile
    DMAs (fast start), blocks 1-3 as one DMA each; weights in 2 DMAs with
    everything feat_q(0)/v need in the first; output staged per 2 chunks
    ([128, 2, 512] f16). HWDGE descriptor generation costs ~625ns per DMA,
    so few big DMAs beat many small ones.
  * 24 standalone LDWEIGHTS in the dead startup window (PE ready ~7us,
    first data ~9.5us). Zero memory side effects (no psum writes, no
    accumulation groups). Measured-neutral-to-slightly-positive; NOTE that
    extending the burst showed the p-state ramp responds to MATMUL
    activity, not LDWEIGHTS, so this is not a reliable clock warm-up.
    Dummy-MATMUL warm-up DID pre-ramp the clock (~2.5us) but exposed a
    timing-sensitive sync hazard on real hardware (deterministic 3.5e-2
    corruption in one configuration, an intermittent NaN in another; the
    instruction-level interpreter reproduces neither) -- do not reintroduce
    without extensive hardware revalidation.
"""

import threading
from contextlib import ExitStack

import numpy as np
import ml_dtypes

import concourse.bass as bass
import concourse.mybir as mybir
import concourse.tile as tile
from concourse import bacc
from concourse.bass_utils import run_bass_kernel_spmd

DIM, HEADS, FEAT = 512, 8, 128
HD = DIM // HEADS          # 64
N = 2048
C = 128                    # chunk (tokens)
NCH = N // C               # 16
NBLK = 4                   # token blocks of 512 for phase A
KT = 4                     # k-tiles of 128 over DIM

F32 = mybir.dt.float32
BF16 = mybir.dt.bfloat16
F16 = mybir.dt.float16
AF = mybir.ActivationFunctionType
ALU = mybir.AluOpType

NP_BF16 = ml_dtypes.bfloat16

# wcat column layout: [wqp(4*128) | wv(4*64) | wkp(4*128) | mask(128) | id(128)]
# first DMA covers wqp+wv (everything feat_q(0) and the v-matmuls need)
WQP0 = 0
WV0 = KT * FEAT                 # 512
WKP0 = WV0 + KT * HD            # 768
WCAT1 = WKP0                    # first-DMA column count
MASK0 = WKP0 + KT * FEAT        # 1280
ID0 = MASK0 + 128               # 1408
WCAT_COLS = ID0 + 128           # 1536



def build_nc():
    nc = bacc.Bacc()

    xT_d = nc.declare_dram_parameter("xt", [128, NBLK, KT, 512], BF16,
                                     isOutput=False)
    wcat_d = nc.declare_dram_parameter("wcat", [128, WCAT_COLS], BF16,
                                       isOutput=False)
    wo_d = nc.declare_dram_parameter("w_out_h", [HD, DIM], BF16, isOutput=False)
    # out[p, c, col] = full_out[token = c*128 + p, col]
    out_d = nc.declare_dram_parameter("out_part", [128, NCH, DIM], F16,
                                      isOutput=True)

    with ExitStack() as ctx:
        tc = ctx.enter_context(tile.TileContext(nc))
        const = ctx.enter_context(tc.tile_pool(name="const", bufs=1))
        fpool = ctx.enter_context(tc.tile_pool(name="feat", bufs=3))
        spool = ctx.enter_context(tc.tile_pool(name="spool", bufs=3))
        ampool = ctx.enter_context(tc.tile_pool(name="am", bufs=8))
        numpool = ctx.enter_context(tc.tile_pool(name="num", bufs=3))
        opool = ctx.enter_context(tc.tile_pool(name="osb", bufs=3))
        dpool = ctx.enter_context(tc.tile_pool(name="dinv", bufs=3))
        # PSUM banks: psml(3) + feat(2) + pj(2) + psp(1) = 8
        psml = ctx.enter_context(tc.tile_pool(name="psml", bufs=3, space="PSUM"))
        pbig = ctx.enter_context(tc.tile_pool(name="pbig", bufs=2, space="PSUM"))
        psp = ctx.enter_context(tc.tile_pool(name="psp", bufs=1, space="PSUM"))

        ps_s = psp.tile([FEAT, HD + 1], F32)   # persistent S accumulator

        # ---- constants; DMA order = first-use order so feat(0) starts early:
        # wqp, x-block0 k-tiles, wkp+wv+mask+id, x-blocks 1-3, wo ----
        wcat = const.tile([128, WCAT_COLS], BF16)
        wo_sb = const.tile([HD, DIM], BF16)
        nc.sync.dma_start(wcat[:, 0:WCAT1], wcat_d[:, 0:WCAT1])

        def wqp(kk):
            return wcat[:, WQP0 + kk * FEAT: WQP0 + (kk + 1) * FEAT]

        def wkp(kk):
            return wcat[:, WKP0 + kk * FEAT: WKP0 + (kk + 1) * FEAT]

        def wv(kk):
            return wcat[:, WV0 + kk * HD: WV0 + (kk + 1) * HD]

        mask_sb = wcat[:, MASK0:MASK0 + 128]
        id_sb = wcat[:, ID0:ID0 + 128]

        s0_sb = const.tile([FEAT, HD + 1], BF16)
        nc.vector.memset(s0_sb[:], 0.0)
        id_one = id_sb[HD:HD + 1, HD:HD + 1]   # [[1]] at base partition 64

        # ---- persistent intermediates ----
        xsb = const.tile([128, NBLK, KT, 512], BF16)   # all of x^T, pre-tiled
        qpT = const.tile([FEAT, N], BF16)              # elu1p(x @ (Wq pm))^T
        kpT = const.tile([FEAT, N], BF16)
        kp_tm = const.tile([128, N], BF16)             # token-major kp
        vv = const.tile([128, NCH, HD + 1], BF16)      # [v | 1] per chunk
        nc.vector.memset(vv[:, :, HD:HD + 1], 1.0)     # ones column, all chunks

        # x DMAs up front; block 0 split by k-tile so featmuls start early.
        # k-tiles 0-1 go through the scalar queue (2nd HWDGE ring) so their
        # ~0.7us descriptor gens run in parallel with sync's wcat1.
        # (NEVER gpsimd SWDGE here: its SBUF descriptor rings + 16 SDMA
        # fetchers on partitions 0-31 slow every SBUF op ~20%, HW-measured.)
        nc.scalar.dma_start(xsb[:, 0, 0, :], xT_d[:, 0, 0, :])
        nc.scalar.dma_start(xsb[:, 0, 1, :], xT_d[:, 0, 1, :])
        nc.sync.dma_start(xsb[:, 0, 2, :], xT_d[:, 0, 2, :])
        nc.sync.dma_start(xsb[:, 0, 3, :], xT_d[:, 0, 3, :])
        nc.sync.dma_start(wcat[:, WCAT1:], wcat_d[:, WCAT1:])
        for blk in range(1, NBLK):
            nc.sync.dma_start(xsb[:, blk, :, :], xT_d[:, blk, :, :])
        nc.sync.dma_start(wo_sb[:], wo_d[:])

        # ---- emission helpers ----
        # feat is split into the MM+EXP part and the DVE part so the DVE
        # ops can be emitted AFTER the S-chain's critical copies: engine
        # queues are in-order, and a rec s-copy queued behind a 600ns
        # maxadd stalls the next S-update (measured as inflated MM slices)
        feat_dve_pending = {}

        def feat_mm(blk, which):
            sl = slice(blk * 512, (blk + 1) * 512)
            wfn, dstT = ((wqp, qpT), (wkp, kpT))[which]
            ps = pbig.tile([FEAT, 512], F32, tag="fps", name=f"fps{blk}_{which}")
            for kk in range(KT):
                nc.tensor.matmul(ps[:], wfn(kk), xsb[:, blk, kk, :],
                                 start=(kk == 0), stop=(kk == KT - 1))
            e = fpool.tile([FEAT, 512], BF16, tag="e", name=f"e{blk}_{which}")
            nc.scalar.activation(e[:], ps[:], AF.Exp)
            feat_dve_pending[(blk, which)] = (ps, e, dstT, sl)

        def feat_dve(blk, which):
            ps, e, dstT, sl = feat_dve_pending.pop((blk, which))
            r = fpool.tile([FEAT, 512], BF16, tag="r", name=f"r{blk}_{which}")
            nc.vector.tensor_scalar(r[:], ps[:], 0.0, 1.0, ALU.max, ALU.add)
            nc.vector.tensor_tensor(dstT[:, sl], e[:], r[:], ALU.min)

        am_tiles = {}

        def v_chunk(i):
            """Token-major v for chunk i, straight from xT k-tiles. Depends
            only on the x DMA -- used as PE filler between recursion steps."""
            blk, sub = i // 4, i % 4
            cs = slice(sub * C, (sub + 1) * C)
            pv = psml.tile([128, HD], F32, tag="sml", name=f"pv{i}")
            for kk in range(KT):
                nc.tensor.matmul(pv[:], xsb[:, blk, kk, cs], wv(kk),
                                 start=(kk == 0), stop=(kk == KT - 1))
            # ACT copy (not DVE): DVE is the phase-A bottleneck engine
            nc.scalar.activation(vv[:, i, 0:HD], pv[:], AF.Copy)

        def prep_chunk(i):
            """S-independent per-chunk work: kp transpose, masked A^T."""
            ci = slice(i * C, (i + 1) * C)
            pt = psml.tile([128, 128], BF16, tag="sml", name=f"pt{i}")
            nc.tensor.transpose(pt[:], kpT[:, ci], id_sb)
            nc.vector.tensor_copy(kp_tm[:, ci], pt[:])
            pa = psml.tile([128, 128], F32, tag="sml", name=f"pa{i}")
            nc.tensor.matmul(pa[:], kpT[:, ci], qpT[:, ci], start=True, stop=True)
            am = ampool.tile([128, 128], BF16, name=f"am{i}")
            nc.vector.tensor_tensor(am[:], pa[:], mask_sb, ALU.mult)
            am_tiles[i] = am

        osb2 = [None]

        def emit_tail(num, i):
            # whole tail of chunk i runs a full chunk after its num copy:
            # pdc/outproj never sit at the PE queue head waiting on DVE
            pdc = psml.tile([128, 1], BF16, tag="sml", name=f"pdc{i}")
            nc.tensor.transpose(pdc[:], num[HD:HD + 1, :], id_one)
            dinv = dpool.tile([128, 1], F32, name=f"dinv{i}")
            nc.vector.reciprocal(dinv[:], pdc[:])
            pj = pbig.tile([128, DIM], F32, tag="pj", name=f"pj{i}")
            nc.tensor.matmul(pj[:], num[0:HD, :], wo_sb[:], start=True, stop=True)
            if i >= NCH - 2:
                # last two chunks ship individually: chunk 14's DMA starts a
                # drain earlier and the final transfer is only 128KB
                osb = opool.tile([128, 1, DIM], F16, name=f"osb1_{i}")
                nc.scalar.activation(osb[:, 0, :], pj[:], AF.Copy,
                                     scale=dinv[:])
                nc.sync.dma_start(out_d[:, i:i + 1, :], osb[:])
                return
            if i % 2 == 0:
                osb2[0] = opool.tile([128, 2, DIM], F16, name=f"osb2_{i}")
            osb = osb2[0]
            nc.scalar.activation(osb[:, i % 2, :], pj[:], AF.Copy,
                                 scale=dinv[:])
            if i % 2 == 1:
                nc.sync.dma_start(out_d[:, i - 1:i + 1, :], osb[:])

        state = {"s_prev": s0_sb, "pending": None}

        def rec_chunk(i):
            """S-chain per-chunk work + pipelined tail of chunk i-1."""
            blk, sub = i // 4, i % 4
            ci = slice(i * C, (i + 1) * C)
            cs = slice(sub * C, (sub + 1) * C)
            s_prev = state["s_prev"]
            am = am_tiles.pop(i)
            # numerator psum tile; row 64 = denominator^T
            pon = psml.tile([HD + 1, 128], F32, tag="sml", name=f"pk{i}")
            po65 = pon[:]
            # S' += kp_tm^T @ [v|1]  (PSUM accumulation across chunks)
            nc.tensor.matmul(ps_s[:], kp_tm[:, ci], vv[:, i, :],
                             start=(i == 0), stop=(i == NCH - 1),
                             skip_group_check=True)
            # [num^T; den^T] [65, ti] = [S|z]^T qpc + [v|1]^T am
            nc.tensor.matmul(po65, s_prev[:], qpT[:, ci],
                             start=True, stop=False, skip_group_check=True)
            nc.tensor.matmul(po65, vv[:, i, :], am[:],
                             start=False, stop=True, skip_group_check=True)
            # num copy first (feeds the next tail's pdc), then S snapshot
            # (DVE for both: keeps the S-chain off the drain-heavy ACT queue)
            num = numpool.tile([HD + 1, 128], BF16, name=f"num{i}")
            nc.vector.tensor_copy(num[:], po65)
            s_new = spool.tile([FEAT, HD + 1], BF16, name=f"s{i}")
            nc.vector.tensor_copy(s_new[:], ps_s[:])
            # previous chunk's tail here: covers num-copy latency
            if state["pending"] is not None:
                emit_tail(*state["pending"])
            state["pending"] = (num, i)
            state["s_prev"] = s_new

        # ---- emission schedule (lag-2): the S-recursion of chunk i runs
        # two chunks behind its prep, so only TWO bare rec steps remain at
        # the end (the bare-rec tail is where the PE goes gappy and the HAM
        # clock-gate re-throttles to 1.2GHz -- measured 537ns N=512 MMs).
        # v/feat/prep matmuls are interleaved as filler between rec steps
        # so the PE stays dense while DVE/ACT service the S-chain. ----
        feat_mm(0, 0)
        v_chunk(0)
        v_chunk(1)
        feat_dve(0, 0)
        v_chunk(2)
        v_chunk(3)
        feat_mm(0, 1)
        feat_dve(0, 1)
        prep_chunk(0)
        prep_chunk(1)
        rec_chunk(0)
        prep_chunk(2)
        rec_chunk(1)
        prep_chunk(3)
        for blk in range(1, NBLK):
            feat_mm(blk, 0)
            rec_chunk(4 * blk - 2)
            v_chunk(4 * blk + 0)
            rec_chunk(4 * blk - 1)
            v_chunk(4 * blk + 1)
            feat_dve(blk, 0)
            feat_mm(blk, 1)
            feat_dve(blk, 1)
            prep_chunk(4 * blk + 0)
            prep_chunk(4 * blk + 1)
            rec_chunk(4 * blk + 0)
            v_chunk(4 * blk + 2)
            rec_chunk(4 * blk + 1)
            v_chunk(4 * blk + 3)
            prep_chunk(4 * blk + 2)
            prep_chunk(4 * blk + 3)
        rec_chunk(14)
        rec_chunk(15)
        emit_tail(*state["pending"])

    nc.compile()
    return nc


_cache = threading.Lock()
_nc = None


def _get_nc():
    global _nc
    with _cache:
        if _nc is None:
            _nc = build_nc()
    return _nc


def _in_maps(x, proj_matrix, W_qkv, W_out):
    # x^T pre-tiled: [512, 2048] -> [kt, 128, blk, 512] -> [128, blk, kt, 512]
    xT = np.ascontiguousarray(
        x[0].T.reshape(KT, 128, NBLK, 512).transpose(1, 2, 0, 3)
    ).astype(NP_BF16)
    mask = (np.arange(128)[:, None] <= np.arange(128)[None, :]).astype(np.float32)
    ident = np.eye(128, dtype=np.float32)

    def ktile(cols):
        # (512, m) -> (128, 4*m) k-tile layout, kt-major columns
        m = cols.shape[1]
        return cols.reshape(KT, 128, m).transpose(1, 0, 2).reshape(128, KT * m)

    maps = []
    for c in range(HEADS):
        pm = proj_matrix[c]                                 # (64, 128)
        wq = W_qkv[:, c * HD:(c + 1) * HD]                  # (512, 64)
        wk = W_qkv[:, DIM + c * HD: DIM + (c + 1) * HD]
        wv_ = W_qkv[:, 2 * DIM + c * HD: 2 * DIM + (c + 1) * HD]
        wcat = np.concatenate(
            [ktile(wq @ pm), ktile(wv_), ktile(wk @ pm), mask, ident],
            axis=1).astype(NP_BF16)
        maps.append({
            "xt": xT,
            "wcat": np.ascontiguousarray(wcat),
            "w_out_h": np.ascontiguousarray(
                W_out[c * HD:(c + 1) * HD, :]).astype(NP_BF16),
        })
    return maps


def kernel(x, proj_matrix, W_qkv, W_out, b_out, _trace=False):
    x = np.asarray(x, dtype=np.float32)
    proj_matrix = np.asarray(proj_matrix, dtype=np.float32)
    W_qkv = np.asarray(W_qkv, dtype=np.float32)
    W_out = np.asarray(W_out, dtype=np.float32)
    b_out = np.asarray(b_out, dtype=np.float32)

    nc = _get_nc()
    maps = _in_maps(x, proj_matrix, W_qkv, W_out)
    res = run_bass_kernel_spmd(nc, maps, core_ids=list(range(HEADS)), trace=_trace)
    out = np.zeros((N, DIM), dtype=np.float32)
    for r in res.results:
        part = np.asarray(r["out_part"], dtype=np.float32)   # [128, 16, 512]
        out += part.transpose(1, 0, 2).reshape(N, DIM)
    out += b_out
    if _trace:
        return out.reshape(1, N, DIM), res
    return out.reshape(1, N, DIM)

